# revision 20
# baseline (speedup 1.0000x reference)
"""MPNN layer (NNConv-style) Trainium2 Bass kernel.

Strategy: shard by destination-node range. Core c owns nodes
[c*6250, (c+1)*6250) and every edge whose dst lands there, so no
cross-core reduction is needed. On the host we lay each core's edges
out into fixed-capacity slots grouped by 128-node destination block
(sorted layout -> segment-sum becomes a one-hot matmul accumulated in
PSUM). On device, per 128-slot tile:
  gather ef rows + nf[src] rows (indirect DMA)
  h^T = relu(W1^T @ ef^T + b1)            (PE + ACT)
  We  = h @ W2perm  (per-edge 32x32, o-major cols) (PE, 2x 512-wide)
  msg = reduce_i(We * x_bcast) + x @ B    (DVE mult+reduce, PE bias mm)
  agg_block += onehot(dst)^T @ msg        (PE, PSUM accumulate)
"""

import sys

for _p in ("/opt/trn_rl_repo",):
    if _p not in sys.path:
        sys.path.insert(0, _p)

import numpy as np

N_NODES = 50000
N_EDGES = 200000
HID = 32
ED = 16
EH = 128
NCORES = 8
NPC = N_NODES // NCORES  # 6250 nodes per core
NBLK = (NPC + 127) // 128  # 49 destination blocks per core
BLKCAP = 640  # edge-slot capacity per block (5 tiles of 128)
TPB = BLKCAP // 128
NSLOT = NBLK * BLKCAP
NTILE = NSLOT // 128

_prog_cache = {}


def _build_program(nblk=NBLK, tpb=TPB, reps=1):
    import concourse.bass as bass
    import concourse.bacc as bacc
    import concourse.mybir as mybir
    from concourse.tile import TileContext
    from concourse.masks import make_identity

    f32 = mybir.dt.float32
    i32 = mybir.dt.int32
    AF = mybir.ActivationFunctionType
    ALU = mybir.AluOpType
    AX = mybir.AxisListType
    nslot = nblk * tpb * 128

    nc = bacc.Bacc(
        "TRN2", target_bir_lowering=False, debug=False, num_devices=NCORES
    )
    ef_d = nc.dram_tensor("ef", [N_EDGES, ED], f32, kind="ExternalInput")
    nf_d = nc.dram_tensor("nf", [N_NODES, HID], f32, kind="ExternalInput")
    W1_d = nc.dram_tensor("W1", [ED, EH], f32, kind="ExternalInput")
    b1_d = nc.dram_tensor("b1c", [EH, 2], f32, kind="ExternalInput")
    W2p_d = nc.dram_tensor("W2p", [EH, HID * HID], f32, kind="ExternalInput")
    Bm_d = nc.dram_tensor("Bm", [HID, HID], f32, kind="ExternalInput")
    biasr_d = nc.dram_tensor("biasr", [128, HID], f32, kind="ExternalInput")
    ntile = nblk * tpb
    meta_d = nc.dram_tensor("meta", [128, ntile * 4], i32, kind="ExternalInput")
    out_d = nc.dram_tensor("out", [nblk * 128, HID], f32, kind="ExternalOutput")

    with TileContext(nc) as tc:
        with (
            tc.tile_pool(name="const", bufs=1) as cp,
            tc.tile_pool(name="work", bufs=3) as wp,
            tc.tile_pool(name="ps_tr", bufs=3, space="PSUM") as ps_tr,
            tc.tile_pool(name="ps_h", bufs=1, space="PSUM") as ps_h,
            tc.tile_pool(name="ps_we", bufs=3, space="PSUM") as ps_we,
            tc.tile_pool(name="ps_agg", bufs=1, space="PSUM") as ps_agg,
        ):
            W1_sb = cp.tile([ED, EH], f32)
            nc.sync.dma_start(out=W1_sb[:], in_=W1_d[:])
            b1_sb = cp.tile([EH, 2], f32)
            nc.sync.dma_start(out=b1_sb[:], in_=b1_d[:])
            W2p_sb = cp.tile([EH, HID * HID], f32)
            nc.sync.dma_start(out=W2p_sb[:], in_=W2p_d[:])
            Bm_sb = cp.tile([HID, HID], f32)
            nc.sync.dma_start(out=Bm_sb[:], in_=Bm_d[:])
            biasr_sb = cp.tile([128, HID], f32)
            nc.sync.dma_start(out=biasr_sb[:], in_=biasr_d[:])
            meta_sb = cp.tile([128, ntile * 4], i32)
            nc.sync.dma_start(out=meta_sb[:], in_=meta_d[:])
            iota_i = cp.tile([128, 128], i32)
            nc.gpsimd.iota(
                out=iota_i[:], pattern=[[1, 128]], channel_multiplier=0
            )
            iota_sb = cp.tile([128, 128], f32)
            nc.vector.tensor_copy(out=iota_sb[:], in_=iota_i[:])
            ident = cp.tile([128, 128], f32)
            make_identity(nc, ident[:])

            for rep_b in range(reps * nblk):
                b = rep_b % nblk
                agg = ps_agg.tile([128, HID], f32, tag="agg")
                for j in range(tpb):
                    t = b * tpb + j
                    dst_t = wp.tile([128, 1], f32, tag="dst")
                    nc.vector.tensor_copy(
                        out=dst_t[:], in_=meta_sb[:, t * 4 + 2 : t * 4 + 3]
                    )
                    ef_t = wp.tile([128, ED], f32, tag="ef")
                    nc.gpsimd.indirect_dma_start(
                        out=ef_t[:],
                        out_offset=None,
                        in_=ef_d[:],
                        in_offset=bass.IndirectOffsetOnAxis(
                            ap=meta_sb[:, t * 4 : t * 4 + 1], axis=0
                        ),
                    )
                    x_t = wp.tile([128, HID], f32, tag="x")
                    nc.gpsimd.indirect_dma_start(
                        out=x_t[:],
                        out_offset=None,
                        in_=nf_d[:],
                        in_offset=bass.IndirectOffsetOnAxis(
                            ap=meta_sb[:, t * 4 + 1 : t * 4 + 2], axis=0
                        ),
                    )
                    # ef^T via PE transpose, then h^T = relu(W1^T @ ef^T + b1)
                    efT_ps = ps_tr.tile([ED, 128], f32, tag="tr")
                    nc.tensor.transpose(
                        out=efT_ps[:], in_=ef_t[:], identity=ident[:]
                    )
                    efT_sb = wp.tile([ED, 128], f32, tag="efT")
                    nc.scalar.copy(out=efT_sb[:], in_=efT_ps[:])
                    hT_ps = ps_h.tile([EH, 128], f32, tag="h")
                    nc.tensor.matmul(
                        out=hT_ps[:], lhsT=W1_sb[:], rhs=efT_sb[:],
                        start=True, stop=True,
                    )
                    h_sb = wp.tile([EH, 128], f32, tag="hsb")
                    nc.scalar.activation(
                        out=h_sb[:], in_=hT_ps[:], func=AF.Relu,
                        bias=b1_sb[:, 0:1], scale=1.0,
                    )
                    # x^T via PE transpose; bias term x @ B
                    xT_ps = ps_tr.tile([HID, 128], f32, tag="tr")
                    nc.tensor.transpose(
                        out=xT_ps[:], in_=x_t[:], identity=ident[:]
                    )
                    xT_sb = wp.tile([HID, 128], f32, tag="xT")
                    nc.scalar.copy(out=xT_sb[:], in_=xT_ps[:])
                    mb_ps = ps_tr.tile([128, HID], f32, tag="tr")
                    nc.tensor.matmul(
                        out=mb_ps[:], lhsT=xT_sb[:], rhs=Bm_sb[:],
                        start=True, stop=True,
                    )
                    # We halves (o-major cols) + fused mult/reduce einsum
                    msg = wp.tile([128, HID], f32, tag="msg")
                    for hh in range(2):
                        We_ps = ps_we.tile([128, 512], f32, tag="we")
                        nc.tensor.matmul(
                            out=We_ps[:],
                            lhsT=h_sb[:],
                            rhs=W2p_sb[:, hh * 512 : (hh + 1) * 512],
                            start=True, stop=True,
                        )
                        prod = wp.tile([128, 512], f32, tag="prod")
                        xb = x_t[:][:, None, :].to_broadcast([128, 16, HID])
                        nc.vector.tensor_tensor(
                            out=prod[:].rearrange("p (o i) -> p o i", i=HID),
                            in0=We_ps[:].rearrange("p (o i) -> p o i", i=HID),
                            in1=xb,
                            op=ALU.mult,
                        )
                        nc.vector.tensor_reduce(
                            out=msg[:, hh * 16 : (hh + 1) * 16],
                            in_=prod[:].rearrange("p (o i) -> p o i", i=HID),
                            axis=AX.X,
                            op=ALU.add,
                        )
                    msg2 = wp.tile([128, HID], f32, tag="msg2")
                    nc.vector.tensor_tensor(
                        out=msg2[:], in0=msg[:], in1=mb_ps[:], op=ALU.add
                    )
                    # one-hot of dst-in-block, accumulate into block agg
                    S_sb = wp.tile([128, 128], f32, tag="S")
                    nc.vector.tensor_tensor(
                        out=S_sb[:],
                        in0=iota_sb[:],
                        in1=dst_t[:, 0:1].to_broadcast([128, 128]),
                        op=ALU.is_equal,
                    )
                    nc.tensor.matmul(
                        out=agg[:], lhsT=S_sb[:], rhs=msg2[:],
                        start=(j == 0), stop=(j == tpb - 1),
                    )
                ob = wp.tile([128, HID], f32, tag="ob")
                nc.vector.tensor_tensor(
                    out=ob[:], in0=agg[:], in1=biasr_sb[:], op=ALU.add
                )
                nc.sync.dma_start(
                    out=out_d[b * 128 : (b + 1) * 128, :], in_=ob[:]
                )
    nc.compile()
    return nc


def _host_layout(edge_src, edge_dst):
    """Slot layout per core + overflow edge list (rarely non-empty)."""
    metas, overflow = [], []
    core = edge_dst // NPC
    for c in range(NCORES):
        sel = np.nonzero(core == c)[0].astype(np.int64)
        ld = edge_dst[sel].astype(np.int64) - c * NPC
        blk = ld >> 7
        order = np.argsort(blk, kind="stable")
        se, sblk, sld = sel[order], blk[order], ld[order]
        counts = np.bincount(sblk, minlength=NBLK)
        starts = np.concatenate(([0], np.cumsum(counts)[:-1]))
        pos = np.arange(len(se)) - starts[sblk]
        keep = pos < BLKCAP
        slot = sblk[keep] * BLKCAP + pos[keep]
        meta = np.zeros((NSLOT, 4), dtype=np.int32)
        meta[:, 2] = -1
        meta[slot, 0] = se[keep].astype(np.int32)
        meta[slot, 1] = edge_src[se[keep]].astype(np.int32)
        meta[slot, 2] = (sld[keep] & 127).astype(np.int32)
        meta_r = np.ascontiguousarray(
            meta.reshape(NTILE, 128, 4).transpose(1, 0, 2).reshape(128, -1)
        )
        metas.append(meta_r)
        overflow.extend(se[~keep].tolist())
    return metas, overflow


def _make_in_maps(nf, ef, edge_src, edge_dst, W1, b1, W2, b2, bias):
    metas, overflow = _host_layout(edge_src, edge_dst)
    W2p = np.ascontiguousarray(
        W2.reshape(EH, HID, HID).transpose(0, 2, 1).reshape(EH, HID * HID)
    )
    common = {
        "ef": ef,
        "nf": nf,
        "W1": W1,
        "b1c": np.ascontiguousarray(np.tile(b1.reshape(EH, 1), (1, 2))),
        "W2p": W2p,
        "Bm": np.ascontiguousarray(b2.reshape(HID, HID)),
        "biasr": np.ascontiguousarray(np.tile(bias[None, :], (128, 1))),
    }
    in_maps = [{**common, "meta": metas[c]} for c in range(NCORES)]
    return in_maps, overflow


def kernel(nf, ef, edge_src, edge_dst, W1, b1, W2, b2, bias):
    from concourse.bass_utils import run_bass_kernel_spmd

    nf = np.asarray(nf, dtype=np.float32)
    ef = np.asarray(ef, dtype=np.float32)
    edge_src = np.asarray(edge_src, dtype=np.int32)
    edge_dst = np.asarray(edge_dst, dtype=np.int32)
    W1 = np.asarray(W1, dtype=np.float32)
    b1 = np.asarray(b1, dtype=np.float32)
    W2 = np.asarray(W2, dtype=np.float32)
    b2 = np.asarray(b2, dtype=np.float32)
    bias = np.asarray(bias, dtype=np.float32)

    if "prog" not in _prog_cache:
        _prog_cache["prog"] = _build_program()
    nc = _prog_cache["prog"]

    in_maps, overflow = _make_in_maps(
        nf, ef, edge_src, edge_dst, W1, b1, W2, b2, bias
    )

    res = run_bass_kernel_spmd(nc, in_maps, core_ids=list(range(NCORES)))
    out = np.concatenate(
        [res.results[c]["out"][:NPC] for c in range(NCORES)], axis=0
    )

    if overflow:  # capacity spill: finish the stragglers on host
        e = np.asarray(overflow, dtype=np.int64)
        h = np.maximum(ef[e] @ W1 + b1, 0.0)
        We = (h @ W2 + b2).reshape(-1, HID, HID)
        msg = np.einsum("ei,eio->eo", nf[edge_src[e]], We)
        np.add.at(out, edge_dst[e], msg)

    return np.ascontiguousarray(out, dtype=np.float32)


# revision 33
# speedup vs baseline: 18.2200x; 18.2200x over previous
"""MPNN layer (NNConv-style) Trainium2 Bass kernel.

Strategy: shard by destination-node range. Core c owns nodes
[c*6250, (c+1)*6250) and every edge whose dst lands there, so no
cross-core reduction is needed. The host lays each core's edge slice
out into fixed-capacity slots grouped by 128-node destination block
(sorted layout -> segment-sum becomes a one-hot matmul accumulated in
PSUM) and ships ef pre-transposed in slot order (each device holds its
edge slice of ef; node features stay replicated and are gathered on
device). bf16 feature path, f32 PSUM accumulation. Per 128-slot tile:
  x = nf[src]                     (indirect DMA gather, bf16 64B rows)
  h^T = relu(W1^T @ ef^T + b1)    (PE masked-K matmul + ACT relu)
  We  = h @ W2perm  (per-edge 32x32, o-major col layout) (PE, 2x512)
  We -> bf16 SBUF                 (ACT copy)
  msg = reduce_i(We * x_bcast)    (DVE bf16 mult + reduce)
  agg_blk  += onehot(dst)^T @ msg (PE, PSUM accumulate)
  aggX_blk += onehot(dst)^T @ x   (PE; bias-term aggregation)
per block: agg += transpose(aggX) @ B; out = agg + bias.
"""

import sys

for _p in ("/opt/trn_rl_repo",):
    if _p not in sys.path:
        sys.path.insert(0, _p)

import numpy as np

N_NODES = 50000
N_EDGES = 200000
HID = 32
ED = 16
EH = 128
NCORES = 8
NPC = N_NODES // NCORES  # 6250 nodes per core
NBLK = (NPC + 127) // 128  # 49 destination blocks per core
BLKCAP = 640  # edge-slot capacity per block (5 tiles of 128)
TPB = BLKCAP // 128
NSLOT = NBLK * BLKCAP
NTILE = NSLOT // 128
NGRP = (NTILE + 3) // 4  # ef^T ships 4 tiles per [128,128] panel

_prog_cache = {}


def _build_program(nblk=NBLK, tpb=TPB, reps=1, skip=()):
    import concourse.bacc as bacc
    import concourse.bass as bass
    import concourse.mybir as mybir
    from concourse.tile import TileContext
    from concourse.masks import make_identity

    f32 = mybir.dt.float32
    bf = mybir.dt.bfloat16
    i32 = mybir.dt.int32
    AF = mybir.ActivationFunctionType
    ALU = mybir.AluOpType
    AX = mybir.AxisListType
    ntile = nblk * tpb
    ngrp = (ntile + 3) // 4

    nc = bacc.Bacc(
        "TRN2", target_bir_lowering=False, debug=False, num_devices=NCORES
    )
    efT_d = nc.dram_tensor("efT4", [128, ngrp * 128], bf, kind="ExternalInput")
    nf_d = nc.dram_tensor("nf16", [N_NODES, HID], bf, kind="ExternalInput")
    W1_d = nc.dram_tensor("W1b", [128, 4 * EH], bf, kind="ExternalInput")
    b1_d = nc.dram_tensor("b1c", [EH, 2], f32, kind="ExternalInput")
    W2p_d = nc.dram_tensor("W2p", [EH, HID * HID], bf, kind="ExternalInput")
    Bm_d = nc.dram_tensor("Bm", [HID, HID], bf, kind="ExternalInput")
    biasr_d = nc.dram_tensor("biasr", [128, HID], f32, kind="ExternalInput")
    meta_d = nc.dram_tensor("meta", [128, ntile * 4], i32, kind="ExternalInput")
    S_d = nc.dram_tensor("Sall", [128, ntile * 128], bf, kind="ExternalInput")
    out_d = nc.dram_tensor("out", [nblk * 128, HID], f32, kind="ExternalOutput")

    with TileContext(nc) as tc:
        with (
            tc.tile_pool(name="const", bufs=1) as cp,
            tc.tile_pool(name="work", bufs=4) as wp,
            tc.tile_pool(name="gath", bufs=3) as gp,
            tc.tile_pool(name="ps_h", bufs=2, space="PSUM") as ps_h,
            tc.tile_pool(name="ps_we", bufs=2, space="PSUM") as ps_we,
            tc.tile_pool(name="ps_agg", bufs=2, space="PSUM") as ps_agg,
            tc.tile_pool(name="ps_ax", bufs=1, space="PSUM") as ps_ax,
            tc.tile_pool(name="ps_tr", bufs=1, space="PSUM") as ps_tr,
        ):
            W1_sb = cp.tile([128, 4 * EH], bf)
            nc.sync.dma_start(out=W1_sb[:], in_=W1_d[:])
            b1_sb = cp.tile([EH, 2], f32)
            nc.sync.dma_start(out=b1_sb[:], in_=b1_d[:])
            W2p_sb = cp.tile([EH, HID * HID], bf)
            nc.sync.dma_start(out=W2p_sb[:], in_=W2p_d[:])
            Bm_sb = cp.tile([HID, HID], bf)
            nc.sync.dma_start(out=Bm_sb[:], in_=Bm_d[:])
            biasr_sb = cp.tile([128, HID], f32)
            nc.sync.dma_start(out=biasr_sb[:], in_=biasr_d[:])
            meta_sb = cp.tile([128, ntile * 4], i32)
            nc.sync.dma_start(out=meta_sb[:], in_=meta_d[:])
            S_all = cp.tile([128, ntile * 128], bf)
            nc.sync.dma_start(out=S_all[:], in_=S_d[:])
            ident = cp.tile([128, 128], bf)
            make_identity(nc, ident[:])

            agg = None
            aggX = None
            for rep in range(reps):
                for g in range(ngrp):
                    tlist = [t for t in range(4 * g, 4 * g + 4) if t < ntile]
                    efT4 = gp.tile([128, 128], bf, tag="efT4")
                    nc.sync.dma_start(
                        out=efT4[:], in_=efT_d[:, g * 128 : (g + 1) * 128]
                    )
                    for c, t in enumerate(tlist):
                        j = t % tpb
                        b = t // tpb
                        x_t = gp.tile([128, HID], bf, tag="x")
                        if "gather" in skip:
                            nc.sync.dma_start(
                                out=x_t[:], in_=nf_d[t * 128 : (t + 1) * 128, :]
                            )
                        else:
                            nc.gpsimd.indirect_dma_start(
                                out=x_t[:],
                                out_offset=None,
                                in_=nf_d[:],
                                in_offset=bass.IndirectOffsetOnAxis(
                                    ap=meta_sb[:, t * 4 + 1 : t * 4 + 2], axis=0
                                ),
                            )
                        hT_ps = ps_h.tile([EH, 128], f32, tag="h")
                        nc.tensor.matmul(
                            out=hT_ps[:],
                            lhsT=W1_sb[:, c * EH : (c + 1) * EH],
                            rhs=efT4[:],
                            start=True, stop=True,
                        )
                        h_sb = wp.tile([EH, 128], bf, tag="hsb")
                        nc.scalar.activation(
                            out=h_sb[:], in_=hT_ps[:], func=AF.Relu,
                            bias=b1_sb[:, 0:1], scale=1.0,
                        )
                        prod = wp.tile([128, HID, HID], bf, tag="prod")
                        for hh in range(2):
                            We_ps = ps_we.tile([128, 512], f32, tag="we")
                            nc.tensor.matmul(
                                out=We_ps[:],
                                lhsT=h_sb[:],
                                rhs=W2p_sb[:, hh * 512 : (hh + 1) * 512],
                                start=True, stop=True,
                            )
                            We_sb = wp.tile([128, 512], bf, tag="wesb")
                            nc.scalar.copy(out=We_sb[:], in_=We_ps[:])
                            xb = x_t[:, None, :].to_broadcast([128, 16, HID])
                            nc.vector.tensor_tensor(
                                out=prod[:, hh * 16 : (hh + 1) * 16, :],
                                in0=We_sb[:].rearrange(
                                    "p (o i) -> p o i", i=HID
                                ),
                                in1=xb,
                                op=ALU.mult,
                            )
                        t1 = wp.tile([128, HID, 16], bf, tag="t1")
                        nc.vector.tensor_tensor(
                            out=t1[:], in0=prod[:, :, 0:16],
                            in1=prod[:, :, 16:32], op=ALU.add,
                        )
                        t2 = wp.tile([128, HID, 8], bf, tag="t2")
                        nc.vector.tensor_tensor(
                            out=t2[:], in0=t1[:, :, 0:8],
                            in1=t1[:, :, 8:16], op=ALU.add,
                        )
                        t3 = wp.tile([128, HID, 4], bf, tag="t3")
                        nc.vector.tensor_tensor(
                            out=t3[:], in0=t2[:, :, 0:4],
                            in1=t2[:, :, 4:8], op=ALU.add,
                        )
                        t4 = wp.tile([128, HID, 2], bf, tag="t4")
                        nc.vector.tensor_tensor(
                            out=t4[:], in0=t3[:, :, 0:2],
                            in1=t3[:, :, 2:4], op=ALU.add,
                        )
                        msg2 = wp.tile([128, HID], bf, tag="msg2")
                        nc.vector.tensor_tensor(
                            out=msg2[:],
                            in0=t4[:, :, 0:1].rearrange("p o one -> p (o one)"),
                            in1=t4[:, :, 1:2].rearrange("p o one -> p (o one)"),
                            op=ALU.add,
                        )
                        if j == 0:
                            agg = ps_agg.tile([128, HID], f32, tag="agg")
                            aggX = ps_ax.tile([128, HID], f32, tag="aggX")
                        nc.tensor.matmul(
                            out=agg[:],
                            lhsT=S_all[:, t * 128 : (t + 1) * 128],
                            rhs=msg2[:],
                            start=(j == 0), stop=False,
                        )
                        nc.tensor.matmul(
                            out=aggX[:],
                            lhsT=S_all[:, t * 128 : (t + 1) * 128],
                            rhs=x_t[:],
                            start=(j == 0), stop=(j == tpb - 1),
                        )
                        if j == tpb - 1:
                            # bias term: agg += aggX @ B  (transpose aggX
                            # on PE, then one K=32 matmul into same bank)
                            aggX_sb = wp.tile([128, HID], bf, tag="axsb")
                            nc.scalar.copy(out=aggX_sb[:], in_=aggX[:])
                            axT_ps = ps_tr.tile([HID, 128], bf, tag="axT")
                            nc.tensor.transpose(
                                out=axT_ps[:], in_=aggX_sb[:],
                                identity=ident[:],
                            )
                            axT_sb = wp.tile([HID, 128], bf, tag="axT_sb")
                            nc.scalar.copy(out=axT_sb[:], in_=axT_ps[:])
                            nc.tensor.matmul(
                                out=agg[:], lhsT=axT_sb[:], rhs=Bm_sb[:],
                                start=False, stop=True,
                            )
                            ob = wp.tile([128, HID], f32, tag="ob")
                            nc.vector.tensor_tensor(
                                out=ob[:], in0=agg[:], in1=biasr_sb[:],
                                op=ALU.add,
                            )
                            nc.sync.dma_start(
                                out=out_d[b * 128 : (b + 1) * 128, :],
                                in_=ob[:],
                            )
    nc.compile()
    return nc


def _host_layout(edge_src, edge_dst):
    """Slot layout per core + overflow edge list (rarely non-empty)."""
    metas, eidxs, overflow = [], [], []
    core = edge_dst // NPC
    for c in range(NCORES):
        sel = np.nonzero(core == c)[0].astype(np.int64)
        ld = edge_dst[sel].astype(np.int64) - c * NPC
        blk = ld >> 7
        order = np.argsort(blk, kind="stable")
        se, sblk, sld = sel[order], blk[order], ld[order]
        counts = np.bincount(sblk, minlength=NBLK)
        starts = np.concatenate(([0], np.cumsum(counts)[:-1]))
        pos = np.arange(len(se)) - starts[sblk]
        keep = pos < BLKCAP
        slot = sblk[keep] * BLKCAP + pos[keep]
        meta = np.zeros((NSLOT, 4), dtype=np.int32)
        meta[:, 2] = -1
        eidx = np.full(NSLOT, -1, dtype=np.int64)
        eidx[slot] = se[keep]
        meta[slot, 0] = se[keep].astype(np.int32)
        meta[slot, 1] = edge_src[se[keep]].astype(np.int32)
        meta[slot, 2] = (sld[keep] & 127).astype(np.int32)
        meta_r = np.ascontiguousarray(
            meta.reshape(NTILE, 128, 4).transpose(1, 0, 2).reshape(128, -1)
        )
        metas.append(meta_r)
        eidxs.append(eidx)
        overflow.extend(se[~keep].tolist())
    return metas, eidxs, overflow


def _make_in_maps(nf, ef, edge_src, edge_dst, W1, b1, W2, b2, bias):
    import ml_dtypes

    bf = ml_dtypes.bfloat16
    metas, eidxs, overflow = _host_layout(edge_src, edge_dst)
    ef_bf = ef.astype(bf)
    W2p = np.ascontiguousarray(
        W2.reshape(EH, HID, HID).transpose(0, 2, 1).reshape(EH, HID * HID)
    ).astype(bf)
    W1r = np.zeros((128, 4 * EH), dtype=bf)
    for c in range(4):
        W1r[c * 32 : c * 32 + ED, c * EH : (c + 1) * EH] = W1.astype(bf)
    common = {
        "nf16": nf.astype(bf),
        "W1b": W1r,
        "b1c": np.ascontiguousarray(np.tile(b1.reshape(EH, 1), (1, 2))),
        "W2p": W2p,
        "Bm": np.ascontiguousarray(b2.reshape(HID, HID)).astype(bf),
        "biasr": np.ascontiguousarray(np.tile(bias[None, :], (128, 1))),
    }
    in_maps = []
    for c in range(NCORES):
        dst_cols = metas[c].reshape(128, NTILE, 4)[:, :, 2]  # [128, NTILE]
        S_nt = np.zeros((128, NTILE, 128), dtype=bf)
        pp, tt = np.nonzero(dst_cols >= 0)
        S_nt[pp, tt, dst_cols[pp, tt]] = 1
        S_all = np.ascontiguousarray(S_nt.reshape(128, NTILE * 128))
        # ef slice in slot order, transposed, packed 4 tiles per panel
        # (tile 4g+q at rows 32q..32q+16 of panel g)
        e_slots = np.zeros((NGRP * 4 * 128, ED), dtype=bf)
        eidx = eidxs[c]
        valid = eidx >= 0
        sl = e_slots[:NSLOT]
        sl[valid] = ef_bf[eidx[valid]]
        et = e_slots.reshape(NGRP, 4, 128, ED).transpose(0, 1, 3, 2)
        efT4 = np.zeros((128, NGRP, 128), dtype=bf)
        for q in range(4):
            efT4[32 * q : 32 * q + ED] = et[:, q].transpose(1, 0, 2)
        efT4 = np.ascontiguousarray(efT4.reshape(128, NGRP * 128))
        in_maps.append(
            {**common, "efT4": efT4, "meta": metas[c], "Sall": S_all}
        )
    return in_maps, overflow


def kernel(nf, ef, edge_src, edge_dst, W1, b1, W2, b2, bias):
    from concourse.bass_utils import run_bass_kernel_spmd

    nf = np.asarray(nf, dtype=np.float32)
    ef = np.asarray(ef, dtype=np.float32)
    edge_src = np.asarray(edge_src, dtype=np.int32)
    edge_dst = np.asarray(edge_dst, dtype=np.int32)
    W1 = np.asarray(W1, dtype=np.float32)
    b1 = np.asarray(b1, dtype=np.float32)
    W2 = np.asarray(W2, dtype=np.float32)
    b2 = np.asarray(b2, dtype=np.float32)
    bias = np.asarray(bias, dtype=np.float32)

    if "prog" not in _prog_cache:
        _prog_cache["prog"] = _build_program()
    nc = _prog_cache["prog"]

    in_maps, overflow = _make_in_maps(
        nf, ef, edge_src, edge_dst, W1, b1, W2, b2, bias
    )

    res = run_bass_kernel_spmd(nc, in_maps, core_ids=list(range(NCORES)))
    out = np.concatenate(
        [res.results[c]["out"][:NPC] for c in range(NCORES)], axis=0
    )

    if overflow:  # capacity spill: finish the stragglers on host
        e = np.asarray(overflow, dtype=np.int64)
        h = np.maximum(ef[e] @ W1 + b1, 0.0)
        We = (h @ W2 + b2).reshape(-1, HID, HID)
        msg = np.einsum("ei,eio->eo", nf[edge_src[e]], We)
        np.add.at(out, edge_dst[e], msg)

    return np.ascontiguousarray(out, dtype=np.float32)
